# revision 40
# baseline (speedup 1.0000x reference)
"""Causal self-attention (B=1, T=4096, C=768, H=12, D=64) on 8 TRN2 NeuronCores.

Sharding: 4 head-groups x 2 query-parity sets (core c: group g=c//2 owning
heads 3g..3g+2, parity qh=c%2 owning query blocks {2j+qh}).  The host sums
the 4 head-group output partials per parity, adds b_out, and reassembles
the interleaved rows.  All SPMD cores run one identical program; per-core
variation enters only through data.

Design (vs the fp32r baseline, 484us -> 264us -> 220us):
  - all PE operands are bf16 (PSUM stays fp32); x pre-transposed/cast on
    host; v produced directly in [t,d] layout; weights host-packed.
    x ships chunk-major ([128, chunk, KO, 512]) so every chunk DMA is one
    contiguous run per partition: 128 descriptors, ~0.6us of issuing-
    engine time instead of >2us (strided-layout DMAs were serializing the
    scalar engine against the exp stream).
  - 256-query supertiles; steady kts (0..4s-1) full width; the 4 tail
    kts trimmed: (kt,block) pairs invalid for BOTH parities are never
    computed or exp'd.  Tail layout per head: bank A = [kt4s q256 |
    kt4s+2 j1 128], bank B = [kt4s+1 q256 | kt4s+3 j1 128]; one
    rectangular ACT [128,2,384] per tile.  Causal masking is a
    multiplicative 0/1 bf16 mask applied by DVE post-exp (per-parity
    tmask data keeps the SPMD program uniform) - no PE mask matmuls.
  - pair unit (heads 0,1): score matmuls row-tiled (K=64 at bases 0/64)
    so both heads run concurrently.  solo unit (head 2): kts row-tiled
    by parity (even kt at rows 0:64, odd at 64:128) with bank-alternated
    PSUM targets so consecutive kts also run concurrently; k1/q1
    projections twin-write rows 64:128 via a concurrent col-tiled matmul
    at tile_position (0,64) (free) to provide the duplicated operands.
    The twin's first matmul needs start=True: the pending-zero clear is
    per-partition, and stale has_written bits would otherwise accumulate
    garbage into the duplicates.
  - startup: ~20 ident warmup matmuls keep the PE (and its HAM clock
    gate) busy while the first x DMAs stream on the sync ring.
  - PSUM: score tiles 2x2-bank slots; 1-bank users share a 2-slot aux
    pool; pair units share one a_ps bank (h0 cols 0:256, h1 256:512).
  - softmax denominators ride the ones-column in the PV stationary
    (a_ps row 64); bf16 DVE reciprocal; bf16 K=1 broadcast matmul; DVE
    multiply.  Host accumulates the 4 head-group partials in fp32.
"""

import numpy as np
import ml_dtypes
from contextlib import ExitStack

import concourse.bass as bass  # noqa: F401
import concourse.mybir as mybir
import concourse.tile as tile
from concourse import bacc
from concourse import bass_utils
from concourse.masks import make_identity
from concourse.tile_rust import add_dep_helper

T, C, H, D = 4096, 768, 12, 64
N_CORES = 8
HPG = 3                    # heads per group
GCH = HPG * D              # 192 channels per group per tensor
TQ = T // 2                # 2048 query rows per core
NTT = T // 128             # 32 key tiles
KO = C // 128              # 6 contraction subtiles
NS = TQ // 256             # 8 query supertiles per core (256 q each)
N_WARM = 20                # HAM warmup matmuls during initial DMA wait

F32 = mybir.dt.float32
F32R = mybir.dt.float32r
BF16 = mybir.dt.bfloat16
AF = mybir.ActivationFunctionType
ALU = mybir.AluOpType

_CACHE = {}
import os
_NODEPS = os.environ.get("BISECT_NODEPS", "0") == "1"
_NOWARM = os.environ.get("K_NOWARM", "0") == "1"
_NOROWTILE = os.environ.get("K_NOROWTILE", "0") == "1"


def build_nc():
    nc = bacc.Bacc(
        "TRN2", target_bir_lowering=False, debug=False, num_devices=N_CORES
    )

    # x ships host-transposed AND chunk-major ([128, chunk, KO, 512]) so each
    # chunk DMA is one contiguous run per partition (128 descriptors, not 768
    # — a strided chunk DMA costs >2us of issuing-engine time).
    xT_d = nc.dram_tensor("xT", [128, 8, KO, 512], BF16, kind="ExternalInput").ap()
    xqT_d = nc.dram_tensor("xqT", [128, 4, KO, 512], BF16, kind="ExternalInput").ap()
    # packed qkv weights: per-ko concat [wk2|wk1|wv3|wq2|wq1] = 576 cols
    wp_d = nc.dram_tensor("wpack", [128, KO * 576], BF16, kind="ExternalInput").ap()
    wo_d = nc.dram_tensor("wo", [GCH, C], BF16, kind="ExternalInput").ap()
    tm_d = nc.dram_tensor("tmask", [128, 2, 384], BF16, kind="ExternalInput").ap()
    out = nc.dram_tensor("out", [C, TQ], BF16, kind="ExternalOutput").ap()

    with tile.TileContext(nc) as tc, ExitStack() as ctx:
        wpool = ctx.enter_context(tc.tile_pool(name="weights", bufs=1))
        dpool = ctx.enter_context(tc.tile_pool(name="data", bufs=1))

        # --- weights / constants ---
        wp_sb = wpool.tile([128, KO, 576], BF16, name="wp_sb")
        nc.sync.dma_start(wp_sb[:], wp_d.rearrange("p (ko n) -> p ko n", n=576))
        W_K2, W_K1, W_V3, W_Q2, W_Q1 = (
            (0, 128), (128, 192), (192, 384), (384, 512), (512, 576)
        )
        wo_sb = [wpool.tile([64, C], BF16, name=f"wo{h}") for h in range(HPG)]
        tm_sb = wpool.tile([128, 2, 384], BF16, name="tm_sb")

        ident32 = wpool.tile([128, 128], F32, name="ident32")
        make_identity(nc, ident32[:])
        ident = wpool.tile([128, 128], BF16, name="ident")
        nc.vector.tensor_copy(ident[:], ident32[:])
        ones65_32 = wpool.tile([65, 64], F32, name="ones65_32")
        nc.vector.memset(ones65_32[:], 1.0)
        ones65 = wpool.tile([65, 64], BF16, name="ones65")
        nc.vector.tensor_copy(ones65[:], ones65_32[:])
        warmz = wpool.tile([128, 512], BF16, name="warmz")
        nc.vector.memset(warmz[:], 0.0)

        # --- persistent tensors ---
        qT2 = dpool.tile([128, TQ], BF16, name="qT2")     # q heads 0,1 [d,t]
        qT1 = dpool.tile([128, TQ], BF16, name="qT1")     # q head 2 (dup rows 64:128)
        kT2 = dpool.tile([128, T], BF16, name="kT2")      # k heads 0,1
        kT1 = dpool.tile([128, T], BF16, name="kT1")      # k head 2 (dup rows 64:128)
        vaug = dpool.tile([128, NTT, HPG, 65], BF16, name="vaug")  # [t,d]+ones
        attnT = [dpool.tile([64, TQ], BF16, name=f"aT{h}") for h in range(HPG)]
        nc.vector.memset(vaug[:, :, :, 64:65], 1.0)

        BK = 2   # kt slots per psum tile
        LAG = 2  # batches between scores and PV
        with (
            tc.tile_pool(name="xchunk", bufs=12) as xpool,
            tc.tile_pool(name="pe", bufs=4 + LAG) as pepool,
            tc.tile_pool(name="rc", bufs=4) as rcpool,
            tc.tile_pool(name="s_ps", bufs=2, space="PSUM") as sps,
            tc.tile_pool(name="a_ps", bufs=2, space="PSUM") as apsp,
            tc.tile_pool(name="x_ps", bufs=2, space="PSUM") as aux,
            tc.tile_pool(name="ob", bufs=3) as ob_pool,
        ):
            # HAM warmup: independent matmuls on constants keep the PE
            # active (and un-throttled) while the first DMAs stream in.
            for i in range(0 if _NOWARM else N_WARM):
                wt = aux.tile([128, 512], F32, tag="aux", name="warm")
                nc.tensor.matmul(wt[:], ident[:], warmz[:], start=True, stop=True)

            xts, xqs = [None] * 8, [None] * 4

            def dma_xt(i, ring, split=False):
                xt = xpool.tile([128, KO, 512], BF16, tag="xt", name=f"xt{i}")
                src = xT_d[:, i, :, :]
                if split:
                    for j in range(KO // 2):
                        ring.dma_start(xt[:, 2 * j : 2 * j + 2, :],
                                       src[:, 2 * j : 2 * j + 2, :])
                else:
                    ring.dma_start(xt[:], src)
                xts[i] = xt

            def dma_xq(c, ring, split=False):
                xq = xpool.tile([128, KO, 512], BF16, tag="xt", name=f"xq{c}")
                src = xqT_d[:, c, :, :]
                if split:
                    for j in range(KO // 2):
                        ring.dma_start(xq[:, 2 * j : 2 * j + 2, :],
                                       src[:, 2 * j : 2 * j + 2, :])
                else:
                    ring.dma_start(xq[:], src)
                xqs[c] = xq

            # first chunks split fine + on the sync ring (starts earliest);
            # the rest stream on the scalar ring in consumption order.
            dma_xt(0, nc.sync, split=True)
            dma_xq(0, nc.sync, split=True)
            nc.sync.dma_start(tm_sb[:], tm_d[:])
            dma_xt(1, nc.sync), dma_xq(1, nc.sync)
            for h in range(HPG):
                nc.sync.dma_start(wo_sb[h][:], wo_d[h * 64 : (h + 1) * 64, :])
            dma_xt(2, nc.scalar), dma_xt(3, nc.scalar), dma_xq(2, nc.scalar)
            dma_xt(4, nc.scalar), dma_xt(5, nc.scalar), dma_xq(3, nc.scalar)
            dma_xt(6, nc.scalar), dma_xt(7, nc.scalar)

            def proj(xt, wcols, m, dest, off, dual=False):
                """dest[:, off:off+512] = wp[:, :, wcols].T @ xt over ko.

                dual: also twin-write rows 64:128 via a concurrent
                col-tiled matmul (same operands, tile_position (0,64))."""
                lo, hi = wcols
                slot = aux.tile([128, 512], F32, tag="aux", name="projps")
                anchor = None
                for ko in range(KO):
                    ma = nc.tensor.matmul(
                        slot[0:m, :],
                        wp_sb[:, ko, lo:hi],
                        xt[:, ko, :],
                        start=(ko == 0),
                        stop=(ko == KO - 1),
                    )
                    if dual:
                        # start=True: the pending-zero clear is per-partition,
                        # so this clears only rows 64:128 (stale has_written
                        # there would otherwise accumulate garbage).
                        mb = nc.tensor.matmul(
                            slot[64:128, :],
                            wp_sb[:, ko, lo:hi],
                            xt[:, ko, :],
                            start=(ko == 0),
                            stop=(ko == KO - 1),
                            skip_group_check=True,
                        )
                        if ko == 0:
                            anchor = ma
                            if not _NODEPS:
                                add_dep_helper(mb.ins, ma.ins, False, "dual order")
                if dual:
                    nc.vector.tensor_copy(dest[:, off : off + 512], slot[:, :])
                else:
                    nc.vector.tensor_copy(dest[:, off : off + 512], slot[0:m, :])

            def kv_chunk(tcnk):
                xt = xts[tcnk]
                t0 = tcnk * 512
                proj(xt, W_K2, 128, kT2, t0)
                proj(xt, W_K1, 64, kT1, t0, dual=True)
                # v in [t, d] layout: xT tile stationary, Wv moving;
                # two t-tiles col-packed per psum bank
                for tp2 in range(2):
                    slot = aux.tile([128, 512], F32, tag="aux", name="vtps")
                    anchor = None
                    for i in range(2):
                        tt = tp2 * 2 + i
                        vt = slot[:, i * 256 : i * 256 + GCH]
                        for ko in range(KO):
                            m = nc.tensor.matmul(
                                vt,
                                xt[:, ko, tt * 128 : (tt + 1) * 128],
                                wp_sb[:, ko, W_V3[0] : W_V3[1]],
                                start=(ko == 0 and i == 0),
                                stop=(ko == KO - 1),
                                skip_group_check=(i == 1),
                            )
                            if ko == 0:
                                if i == 0:
                                    anchor = m
                                elif not _NODEPS:
                                    add_dep_helper(m.ins, anchor.ins, False, "vt order")
                    gt = tcnk * 4 + tp2 * 2
                    nc.vector.tensor_copy(
                        vaug[:, gt : gt + 2, :, 0:64],
                        slot[:, :].rearrange("p (i x) -> p i x", x=256)[
                            :, :, 0:GCH
                        ].rearrange("p i (h d) -> p i h d", h=HPG),
                    )

            def q_chunk(c):
                proj(xqs[c], W_Q2, 128, qT2, c * 512)
                proj(xqs[c], W_Q1, 64, qT1, c * 512, dual=True)

            def s_lhsT(h, kt, rb=0):
                ksl = slice(kt * 128, (kt + 1) * 128)
                if h == 0:
                    return kT2[0:64, ksl]
                if h == 1:
                    return kT2[64:128, ksl]
                return kT1[rb : rb + 64, ksl]

            def s_rhs(h, s, q0, n, rb=0):
                qsl = slice(s * 256 + q0, s * 256 + q0 + n)
                if h == 0:
                    return qT2[0:64, qsl]
                if h == 1:
                    return qT2[64:128, qsl]
                return qT1[rb : rb + 64, qsl]

            def start_norm(kind, s, a_ps):
                w = 512 if kind == "pair" else 256
                an = rcpool.tile([65, 512], F32, tag="an")
                nc.vector.tensor_copy(an[:, 0:w], a_ps[0:65, 0:w])
                # reciprocal on bf16: eligible for the DVE 2x_1P perf mode
                # (fp32 iterative divide runs 1x at ~8 cyc/elem)
                rcb = rcpool.tile([65, 512], BF16, tag="rcb")
                nc.vector.tensor_copy(rcb[64:65, 0:w], an[64:65, 0:w])
                with nc.allow_low_precision(reason="bf16 reciprocal of softmax denom"):
                    nc.vector.reciprocal(rcb[64:65, 0:w], rcb[64:65, 0:w])
                return (kind, s, an, rcb)

            def finish_norm(kind, s, an, rcb):
                qsl = slice(s * 256, (s + 1) * 256)
                w = 512 if kind == "pair" else 256
                r_ps = aux.tile([128, 512], F32, tag="aux", name="rep")[0:64, :]
                nc.tensor.matmul(
                    r_ps[:, 0:w],
                    ones65[64:65, :],
                    rcb[64:65, 0:w],
                    start=True,
                    stop=True,
                )
                hs = (0, 1) if kind == "pair" else (2,)
                for i, h in enumerate(hs):
                    nc.vector.tensor_tensor(
                        attnT[h][:, qsl],
                        an[0:64, i * 256 : (i + 1) * 256],
                        r_ps[:, i * 256 : (i + 1) * 256],
                        ALU.mult,
                    )

            # pipeline state
            pend_pv = []    # (a_ps, pe_t, pv_ops, pv_first, norm_args)
            pend_norm = []  # (due_batch, norm_args)
            batch_no = [0]

            def flush_pv(keep):
                while len(pend_pv) > keep:
                    a_ps, pe_t, ops, pv_first, norm_after = pend_pv.pop(0)
                    for h, kt, j, pc, oc, n, last in ops:
                        first = (kt == 0) and not pv_first
                        m = nc.tensor.matmul(
                            a_ps[0:65, oc : oc + n],
                            vaug[:, kt, h, 0:65],
                            pe_t[:, j, pc : pc + n],
                            start=first,
                            stop=last,
                            skip_group_check=not first,
                        )
                        if first:
                            pv_first.append(m)
                        elif kt == 0 and not _NODEPS:
                            add_dep_helper(m.ins, pv_first[0].ins, False, "aps order")
                    if norm_after is not None:
                        pend_norm.append(
                            (batch_no[0] + 4, start_norm(*norm_after))
                        )

            def flush_norms(force=False):
                while pend_norm and (force or pend_norm[0][0] <= batch_no[0]):
                    _, args = pend_norm.pop(0)
                    finish_norm(*args)

            def emit_phaseD(ts, half=None):
                # half: 0/1 selects a 256-query half of the ts chunk (used to
                # pull the s=6 half of the final chunk out of the tail)
                if half is None:
                    tsl = slice(ts * 512, (ts + 1) * 512)
                    w = 512
                else:
                    tsl = slice(ts * 512 + half * 256, ts * 512 + (half + 1) * 256)
                    w = 256
                for oc in range(C // 128):
                    ocs = slice(oc * 128, (oc + 1) * 128)
                    po = aux.tile([128, 512], F32, tag="aux", name="po")
                    for h in range(HPG):
                        nc.tensor.matmul(
                            po[:, 0:w],
                            wo_sb[h][:, ocs],
                            attnT[h][:, tsl],
                            start=(h == 0),
                            stop=(h == HPG - 1),
                        )
                    ob = ob_pool.tile([128, 512], BF16, tag="ob")
                    # copy via ScalarE: mid-kernel it has slack, and in the
                    # tail the exp stream is already done while DVE paces
                    with nc.allow_low_precision(reason="bf16 output cast"):
                        nc.scalar.copy(ob[:, 0:w], po[:, 0:w])
                    nc.sync.dma_start(out[ocs, tsl], ob[:, 0:w])

            def run_batch(s, kind, a_ps, pv_first, sc_ops, tail, exp_w,
                          pv_ops, norm_after):
                """sc_ops: (h, kt, bank, col, q0, n, rowbase)
                   pv_ops: (h, kt, bank, pecol, outcol, n, last)"""
                bs = sps.tile([128, BK, 512], F32, tag="s")
                bank_first = {}
                for h, kt, j, c0, q0, n, rb in sc_ops:
                    first = j not in bank_first
                    m = nc.tensor.matmul(
                        bs[:, j, c0 : c0 + n],
                        s_lhsT(h, kt, rb), s_rhs(h, s, q0, n, rb),
                        start=first, stop=True,
                        skip_group_check=not first,
                    )
                    if first:
                        bank_first[j] = m
                    elif not _NODEPS:
                        add_dep_helper(m.ins, bank_first[j].ins, False, "bank order")
                batch_no[0] += 1
                flush_pv(LAG)
                flush_norms()
                pe_t = pepool.tile([128, BK, 512], BF16, tag="pe")
                nc.scalar.activation(
                    pe_t[:, :, 0:exp_w], bs[:, :, 0:exp_w], AF.Exp, scale=0.125
                )
                if tail:
                    # multiplicative 0/1 tail mask on DVE (keeps masking off
                    # the PE; diag + invalid blocks zeroed post-exp)
                    nc.vector.tensor_tensor(
                        pe_t[:, :, 0:384], pe_t[:, :, 0:384], tm_sb[:],
                        ALU.mult,
                    )
                pend_pv.append((a_ps, pe_t, pv_ops, pv_first, norm_after))

            def attn_unit(s, kind):
                nkt_steady = 4 * s
                t0 = 4 * s  # first tail kt
                flush_norms(force=True)
                if kind == "solo" and s >= 2 and s % 2 == 0:
                    emit_phaseD((s - 2) // 2)
                if kind == "solo" and s == 7:
                    # s=6 norms are flushed by now: the s=6 half of the final
                    # output chunk can overlap s=7's attention work
                    emit_phaseD(3, half=0)
                w_aps = 512 if kind == "pair" else 256
                a_ps = apsp.tile([65, w_aps], F32, tag="attn", name="a_ps")
                pv_first = []

                if kind == "pair":
                    for kt0 in range(0, nkt_steady, 2):
                        sc, pv = [], []
                        for i, kt in enumerate((kt0, kt0 + 1)):
                            for h in (0, 1):
                                sc.append((h, kt, h, i * 256, 0, 256, 0))
                                pv.append((h, kt, h, i * 256, h * 256, 256, False))
                        run_batch(s, kind, a_ps, pv_first, sc, False, 512, pv, None)
                    # tail: one tile per head; bank0=[kt0 q256|kt0+2 j1],
                    # bank1=[kt0+1 q256|kt0+3 j1]
                    for h in (0, 1):
                        sc = [
                            (h, t0, 0, 0, 0, 256, 0),
                            (h, t0 + 1, 1, 0, 0, 256, 0),
                            (h, t0 + 2, 0, 256, 128, 128, 0),
                            (h, t0 + 3, 1, 256, 128, 128, 0),
                        ]
                        # kt4s+1 last with stop=True: its 256-wide write is
                        # the final touch on every element of this head's
                        # a_ps region, closing the accumulation group.
                        oc = h * 256
                        pv = [
                            (h, t0, 0, 0, oc, 256, False),
                            (h, t0 + 2, 0, 256, oc + 128, 128, False),
                            (h, t0 + 3, 1, 256, oc + 128, 128, False),
                            (h, t0 + 1, 1, 0, oc, 256, True),
                        ]
                        norm = ("pair", s, a_ps) if h == 1 else None
                        run_batch(s, kind, a_ps, pv_first, sc, True, 384, pv, norm)
                else:
                    for kt0 in range(0, nkt_steady, 4):
                        sc, pv = [], []
                        for d, kt in enumerate(range(kt0, kt0 + 4)):
                            j, c0 = d & 1, (d // 2) * 256
                            rb = 0 if _NOROWTILE else 64 * (kt % 2)
                            sc.append((2, kt, j, c0, 0, 256, rb))
                            pv.append((2, kt, j, c0, 0, 256, False))
                        run_batch(s, kind, a_ps, pv_first, sc, False, 512, pv, None)
                    rbo = 0 if _NOROWTILE else 64
                    sc = [
                        (2, t0, 0, 0, 0, 256, 0),
                        (2, t0 + 1, 1, 0, 0, 256, rbo),
                        (2, t0 + 2, 0, 256, 128, 128, 0),
                        (2, t0 + 3, 1, 256, 128, 128, rbo),
                    ]
                    pv = [
                        (2, t0, 0, 0, 0, 256, False),
                        (2, t0 + 2, 0, 256, 128, 128, False),
                        (2, t0 + 3, 1, 256, 128, 128, False),
                        (2, t0 + 1, 1, 0, 0, 256, True),
                    ]
                    run_batch(s, kind, a_ps, pv_first, sc, True, 384, pv,
                              ("solo", s, a_ps))

            unit_list = []
            for s in range(NS):
                unit_list.append(("kv", s))
                if s == 0:
                    unit_list.append(("q", 0))
                unit_list.append((s, "pair"))
                unit_list.append((s, "solo"))
                if s % 2 == 1 and s < 7:
                    unit_list.append(("q", (s + 1) // 2))

            for s, kind in unit_list:
                if s == "kv":
                    kv_chunk(kind)
                    continue
                if s == "q":
                    q_chunk(kind)
                    continue
                attn_unit(s, kind)
            flush_pv(0)
            flush_norms(force=True)
            emit_phaseD(3, half=1)

    nc.compile()
    return nc


def _get_nc():
    if "nc" not in _CACHE:
        _CACHE["nc"] = build_nc()
    return _CACHE["nc"]


BF = ml_dtypes.bfloat16


def pack_w(w):
    """[C, n] -> [128, KO*n] so each SBUF partition row is contiguous."""
    n = w.shape[1]
    return np.ascontiguousarray(
        w.reshape(KO, 128, n).transpose(1, 0, 2).reshape(128, KO * n)
    ).astype(BF)


def make_in_maps(inputs):
    """Shard full inputs into 8 per-core input maps.

    tmask [128k, 2, 384]: per-parity additive masks for the two tail
    banks.  Bank A covers [kt4s j0 | kt4s j1 | kt4s+2 j1], bank B covers
    [kt4s+1 j0 | kt4s+1 j1 | kt4s+3 j1].
      qh=0: A = [diag | 0 | diag],  B = [-inf | 0 | -inf]
      qh=1: A = [0    | 0 | 0   ],  B = [diag | 0 | diag]
    """
    x = np.ascontiguousarray(np.asarray(inputs["x"], dtype=np.float32)).reshape(T, C)
    W_qkv = np.asarray(inputs["W_qkv"], dtype=np.float32)
    W_out = np.asarray(inputs["W_out"], dtype=np.float32)

    # multiplicative masks: keep = 1, drop = 0; [k, q]: keep q >= k
    diag_keep = np.where(
        np.arange(128)[None, :] >= np.arange(128)[:, None],
        np.float32(1), np.float32(0),
    )

    # chunk-major layouts: [128, chunk, KO, 512]
    xT = np.ascontiguousarray(
        x.T.reshape(KO, 128, 8, 512).transpose(1, 2, 0, 3)
    ).astype(BF)
    xr = x.reshape(NTT, 128, C)
    xqT = {
        qh: np.ascontiguousarray(
            xr[qh::2].reshape(TQ, C).T.reshape(KO, 128, 4, 512).transpose(1, 2, 0, 3)
        ).astype(BF)
        for qh in (0, 1)
    }

    ZK = np.zeros((128, 128), np.float32)   # fully-masked block
    ONE = np.ones((128, 128), np.float32)   # fully-valid block
    tmask = {}
    for qh in (0, 1):
        if qh == 0:
            A = np.concatenate([diag_keep, ONE, diag_keep], axis=1)
            B = np.concatenate([ZK, ONE, ZK], axis=1)
        else:
            A = np.concatenate([ONE, ONE, ONE], axis=1)
            B = np.concatenate([diag_keep, ONE, diag_keep], axis=1)
        tmask[qh] = np.stack([A, B], axis=1).astype(BF)  # [128, 2, 384]

    in_maps = []
    for c in range(N_CORES):
        g, qh = c // 2, c % 2
        in_maps.append(
            {
                "xT": xT,
                "xqT": xqT[qh],
                "wpack": pack_w(
                    np.concatenate(
                        [
                            W_qkv[:, 1 * C + g * GCH : 1 * C + (g + 1) * GCH],
                            W_qkv[:, 2 * C + g * GCH : 2 * C + (g + 1) * GCH],
                            W_qkv[:, 0 * C + g * GCH : 0 * C + (g + 1) * GCH],
                        ],
                        axis=1,
                    )
                ),
                "wo": np.ascontiguousarray(W_out[g * GCH : (g + 1) * GCH, :]).astype(BF),
                "tmask": tmask[qh],
            }
        )
    return in_maps


def combine_outputs(parts, b_out):
    """Sum head-group partials per parity, reassemble rows, add bias."""
    NQT = TQ // 128
    out = np.zeros((T, C), np.float32)
    orow = out.reshape(NTT, 128, C)
    for qh in (0, 1):
        acc = parts[qh].astype(np.float32).copy()
        for g in range(1, 4):
            acc += parts[2 * g + qh]
        orow[qh::2] = np.ascontiguousarray(acc.T).reshape(NQT, 128, C)
    out += np.asarray(b_out, dtype=np.float32)[None, :]
    return out.reshape(1, T, C)


def _run(inputs, trace=False, tmpdir=None):
    nc = _get_nc()
    in_maps = make_in_maps(inputs)
    res = bass_utils.run_bass_kernel_spmd(
        nc, in_maps, core_ids=list(range(N_CORES)), trace=trace, tmpdir=tmpdir
    )
    parts = [np.asarray(res.results[c]["out"]) for c in range(N_CORES)]
    return combine_outputs(parts, inputs["b_out"]), res


def kernel(**inputs):
    out, _ = _run(inputs)
    return out


# revision 41
# speedup vs baseline: 1.1887x; 1.1887x over previous
"""Causal self-attention (B=1, T=4096, C=768, H=12, D=64) on 8 TRN2 NeuronCores.

Sharding: 4 head-groups x 2 query-parity sets (core c: group g=c//2 owning
heads 3g..3g+2, parity qh=c%2 owning query blocks {2j+qh}).  The host sums
the 4 head-group output partials per parity, adds b_out, and reassembles
the interleaved rows.  All SPMD cores run one identical program; per-core
variation enters only through data.

Design (vs the fp32r baseline, 484us -> 264us -> 220us):
  - all PE operands are bf16 (PSUM stays fp32); x pre-transposed/cast on
    host; v produced directly in [t,d] layout; weights host-packed.
    x ships chunk-major ([128, chunk, KO, 512]) so every chunk DMA is one
    contiguous run per partition: 128 descriptors, ~0.6us of issuing-
    engine time instead of >2us (strided-layout DMAs were serializing the
    scalar engine against the exp stream).
  - 256-query supertiles; steady kts (0..4s-1) full width; the 4 tail
    kts trimmed: (kt,block) pairs invalid for BOTH parities are never
    computed or exp'd.  Tail layout per head: bank A = [kt4s q256 |
    kt4s+2 j1 128], bank B = [kt4s+1 q256 | kt4s+3 j1 128]; one
    rectangular ACT [128,2,384] per tile.  Causal masking is a
    multiplicative 0/1 bf16 mask applied by DVE post-exp (per-parity
    tmask data keeps the SPMD program uniform) - no PE mask matmuls.
  - pair unit (heads 0,1): score matmuls row-tiled (K=64 at bases 0/64)
    so both heads run concurrently.  solo unit (head 2): kts row-tiled
    by parity (even kt at rows 0:64, odd at 64:128) with bank-alternated
    PSUM targets so consecutive kts also run concurrently; k1/q1
    projections twin-write rows 64:128 via a concurrent col-tiled matmul
    at tile_position (0,64) (free) to provide the duplicated operands.
    The twin's first matmul needs start=True: the pending-zero clear is
    per-partition, and stale has_written bits would otherwise accumulate
    garbage into the duplicates.
  - startup: ~20 ident warmup matmuls keep the PE (and its HAM clock
    gate) busy while the first x DMAs stream on the sync ring.
  - PSUM: score tiles 2x2-bank slots; 1-bank users share a 2-slot aux
    pool; pair units share one a_ps bank (h0 cols 0:256, h1 256:512).
  - softmax denominators ride the ones-column in the PV stationary
    (a_ps row 64); bf16 DVE reciprocal; bf16 K=1 broadcast matmul; DVE
    multiply.  Host accumulates the 4 head-group partials in fp32.
"""

import numpy as np
import ml_dtypes
from contextlib import ExitStack

import concourse.bass as bass  # noqa: F401
import concourse.mybir as mybir
import concourse.tile as tile
from concourse import bacc
from concourse import bass_utils
from concourse.masks import make_identity
from concourse.tile_rust import add_dep_helper

T, C, H, D = 4096, 768, 12, 64
N_CORES = 8
HPG = 3                    # heads per group
GCH = HPG * D              # 192 channels per group per tensor
TQ = T // 2                # 2048 query rows per core
NTT = T // 128             # 32 key tiles
KO = C // 128              # 6 contraction subtiles
NS = TQ // 256             # 8 query supertiles per core (256 q each)
N_WARM = 20                # HAM warmup matmuls during initial DMA wait

F32 = mybir.dt.float32
F32R = mybir.dt.float32r
BF16 = mybir.dt.bfloat16
AF = mybir.ActivationFunctionType
ALU = mybir.AluOpType

_CACHE = {}
import os
_NODEPS = os.environ.get("BISECT_NODEPS", "0") == "1"
_NOWARM = os.environ.get("K_NOWARM", "0") == "1"
_NOROWTILE = os.environ.get("K_NOROWTILE", "0") == "1"


def build_nc():
    nc = bacc.Bacc(
        "TRN2", target_bir_lowering=False, debug=False, num_devices=N_CORES
    )

    # x ships host-transposed AND chunk-major ([128, chunk, KO, 512]) so each
    # chunk DMA is one contiguous run per partition (128 descriptors, not 768
    # — a strided chunk DMA costs >2us of issuing-engine time).
    xT_d = nc.dram_tensor("xT", [128, 8, KO, 512], BF16, kind="ExternalInput").ap()
    xqT_d = nc.dram_tensor("xqT", [128, 4, KO, 512], BF16, kind="ExternalInput").ap()
    # packed qkv weights: per-ko concat [wk2|wk1|wv3|wq2|wq1] = 576 cols
    wp_d = nc.dram_tensor("wpack", [128, KO * 576], BF16, kind="ExternalInput").ap()
    wo_d = nc.dram_tensor("wo", [GCH, C], BF16, kind="ExternalInput").ap()
    tm_d = nc.dram_tensor("tmask", [128, 2, 384], BF16, kind="ExternalInput").ap()
    out = nc.dram_tensor("out", [C, TQ], BF16, kind="ExternalOutput").ap()

    with tile.TileContext(nc) as tc, ExitStack() as ctx:
        wpool = ctx.enter_context(tc.tile_pool(name="weights", bufs=1))
        dpool = ctx.enter_context(tc.tile_pool(name="data", bufs=1))

        # --- weights / constants ---
        wp_sb = wpool.tile([128, KO, 576], BF16, name="wp_sb")
        nc.sync.dma_start(wp_sb[:], wp_d.rearrange("p (ko n) -> p ko n", n=576))
        W_K2, W_K1, W_V3, W_Q2, W_Q1 = (
            (0, 128), (128, 192), (192, 384), (384, 512), (512, 576)
        )
        wo_sb = [wpool.tile([64, C], BF16, name=f"wo{h}") for h in range(HPG)]
        tm_sb = wpool.tile([128, 2, 384], BF16, name="tm_sb")

        ident32 = wpool.tile([128, 128], F32, name="ident32")
        make_identity(nc, ident32[:])
        ident = wpool.tile([128, 128], BF16, name="ident")
        nc.vector.tensor_copy(ident[:], ident32[:])
        ones65_32 = wpool.tile([65, 64], F32, name="ones65_32")
        nc.vector.memset(ones65_32[:], 1.0)
        ones65 = wpool.tile([65, 64], BF16, name="ones65")
        nc.vector.tensor_copy(ones65[:], ones65_32[:])
        warmz = wpool.tile([128, 512], BF16, name="warmz")
        nc.vector.memset(warmz[:], 0.0)

        # --- persistent tensors ---
        qT2 = dpool.tile([128, TQ], BF16, name="qT2")     # q heads 0,1 [d,t]
        qT1 = dpool.tile([128, TQ], BF16, name="qT1")     # q head 2 (dup rows 64:128)
        kT2 = dpool.tile([128, T], BF16, name="kT2")      # k heads 0,1
        kT1 = dpool.tile([128, T], BF16, name="kT1")      # k head 2 (dup rows 64:128)
        vaug = dpool.tile([128, NTT, HPG, 65], BF16, name="vaug")  # [t,d]+ones
        attnT = [dpool.tile([64, TQ], BF16, name=f"aT{h}") for h in range(HPG)]
        nc.vector.memset(vaug[:, :, :, 64:65], 1.0)

        BK = 2   # kt slots per psum tile
        LAG = 2  # batches between scores and PV
        with (
            tc.tile_pool(name="xchunk", bufs=12) as xpool,
            tc.tile_pool(name="pe", bufs=4 + LAG) as pepool,
            tc.tile_pool(name="rc", bufs=4) as rcpool,
            tc.tile_pool(name="s_ps", bufs=2, space="PSUM") as sps,
            tc.tile_pool(name="a_ps", bufs=2, space="PSUM") as apsp,
            tc.tile_pool(name="x_ps", bufs=2, space="PSUM") as aux,
            tc.tile_pool(name="ob", bufs=3) as ob_pool,
        ):
            # HAM warmup: independent matmuls on constants keep the PE
            # active (and un-throttled) while the first DMAs stream in.
            for i in range(0 if _NOWARM else N_WARM):
                wt = aux.tile([128, 512], F32, tag="aux", name="warm")
                nc.tensor.matmul(wt[:], ident[:], warmz[:], start=True, stop=True)

            xts, xqs = [None] * 8, [None] * 4

            def dma_xt(i, ring, split=False):
                xt = xpool.tile([128, KO, 512], BF16, tag="xt", name=f"xt{i}")
                src = xT_d[:, i, :, :]
                if split:
                    for j in range(KO // 2):
                        ring.dma_start(xt[:, 2 * j : 2 * j + 2, :],
                                       src[:, 2 * j : 2 * j + 2, :])
                else:
                    ring.dma_start(xt[:], src)
                xts[i] = xt

            def dma_xq(c, ring, split=False):
                xq = xpool.tile([128, KO, 512], BF16, tag="xt", name=f"xq{c}")
                src = xqT_d[:, c, :, :]
                if split:
                    for j in range(KO // 2):
                        ring.dma_start(xq[:, 2 * j : 2 * j + 2, :],
                                       src[:, 2 * j : 2 * j + 2, :])
                else:
                    ring.dma_start(xq[:], src)
                xqs[c] = xq

            # first chunks split fine + on the sync ring (starts earliest);
            # the rest stream on the scalar ring in consumption order.
            dma_xt(0, nc.sync, split=True)
            dma_xq(0, nc.sync, split=True)
            nc.sync.dma_start(tm_sb[:], tm_d[:])
            dma_xt(1, nc.scalar), dma_xq(1, nc.scalar)
            for h in range(HPG):
                nc.sync.dma_start(wo_sb[h][:], wo_d[h * 64 : (h + 1) * 64, :])
            dma_xt(2, nc.scalar), dma_xt(3, nc.scalar), dma_xq(2, nc.scalar)
            dma_xt(4, nc.scalar), dma_xt(5, nc.scalar), dma_xq(3, nc.scalar)
            dma_xt(6, nc.scalar), dma_xt(7, nc.scalar)

            def proj(xt, wcols, m, dest, off, dual=False):
                """dest[:, off:off+512] = wp[:, :, wcols].T @ xt over ko.

                dual: also twin-write rows 64:128 via a concurrent
                col-tiled matmul (same operands, tile_position (0,64))."""
                lo, hi = wcols
                slot = aux.tile([128, 512], F32, tag="aux", name="projps")
                anchor = None
                for ko in range(KO):
                    ma = nc.tensor.matmul(
                        slot[0:m, :],
                        wp_sb[:, ko, lo:hi],
                        xt[:, ko, :],
                        start=(ko == 0),
                        stop=(ko == KO - 1),
                    )
                    if dual:
                        # start=True: the pending-zero clear is per-partition,
                        # so this clears only rows 64:128 (stale has_written
                        # there would otherwise accumulate garbage).
                        mb = nc.tensor.matmul(
                            slot[64:128, :],
                            wp_sb[:, ko, lo:hi],
                            xt[:, ko, :],
                            start=(ko == 0),
                            stop=(ko == KO - 1),
                            skip_group_check=True,
                        )
                        if ko == 0:
                            anchor = ma
                            if not _NODEPS:
                                add_dep_helper(mb.ins, ma.ins, False, "dual order")
                if dual:
                    nc.vector.tensor_copy(dest[:, off : off + 512], slot[:, :])
                else:
                    nc.vector.tensor_copy(dest[:, off : off + 512], slot[0:m, :])

            def kv_chunk(tcnk):
                xt = xts[tcnk]
                t0 = tcnk * 512
                proj(xt, W_K2, 128, kT2, t0)
                proj(xt, W_K1, 64, kT1, t0, dual=True)
                # v in [t, d] layout: xT tile stationary, Wv moving;
                # two t-tiles col-packed per psum bank
                for tp2 in range(2):
                    slot = aux.tile([128, 512], F32, tag="aux", name="vtps")
                    anchor = None
                    for i in range(2):
                        tt = tp2 * 2 + i
                        vt = slot[:, i * 256 : i * 256 + GCH]
                        for ko in range(KO):
                            m = nc.tensor.matmul(
                                vt,
                                xt[:, ko, tt * 128 : (tt + 1) * 128],
                                wp_sb[:, ko, W_V3[0] : W_V3[1]],
                                start=(ko == 0 and i == 0),
                                stop=(ko == KO - 1),
                                skip_group_check=(i == 1),
                            )
                            if ko == 0:
                                if i == 0:
                                    anchor = m
                                elif not _NODEPS:
                                    add_dep_helper(m.ins, anchor.ins, False, "vt order")
                    gt = tcnk * 4 + tp2 * 2
                    nc.vector.tensor_copy(
                        vaug[:, gt : gt + 2, :, 0:64],
                        slot[:, :].rearrange("p (i x) -> p i x", x=256)[
                            :, :, 0:GCH
                        ].rearrange("p i (h d) -> p i h d", h=HPG),
                    )

            def q_chunk(c):
                proj(xqs[c], W_Q2, 128, qT2, c * 512)
                proj(xqs[c], W_Q1, 64, qT1, c * 512, dual=True)

            def s_lhsT(h, kt, rb=0):
                ksl = slice(kt * 128, (kt + 1) * 128)
                if h == 0:
                    return kT2[0:64, ksl]
                if h == 1:
                    return kT2[64:128, ksl]
                return kT1[rb : rb + 64, ksl]

            def s_rhs(h, s, q0, n, rb=0):
                qsl = slice(s * 256 + q0, s * 256 + q0 + n)
                if h == 0:
                    return qT2[0:64, qsl]
                if h == 1:
                    return qT2[64:128, qsl]
                return qT1[rb : rb + 64, qsl]

            def start_norm(kind, s, a_ps):
                w = 512 if kind == "pair" else 256
                an = rcpool.tile([65, 512], F32, tag="an")
                nc.vector.tensor_copy(an[:, 0:w], a_ps[0:65, 0:w])
                # reciprocal on bf16: eligible for the DVE 2x_1P perf mode
                # (fp32 iterative divide runs 1x at ~8 cyc/elem)
                rcb = rcpool.tile([65, 512], BF16, tag="rcb")
                nc.vector.tensor_copy(rcb[64:65, 0:w], an[64:65, 0:w])
                with nc.allow_low_precision(reason="bf16 reciprocal of softmax denom"):
                    nc.vector.reciprocal(rcb[64:65, 0:w], rcb[64:65, 0:w])
                return (kind, s, an, rcb)

            def finish_norm(kind, s, an, rcb):
                qsl = slice(s * 256, (s + 1) * 256)
                w = 512 if kind == "pair" else 256
                r_ps = aux.tile([128, 512], F32, tag="aux", name="rep")[0:64, :]
                nc.tensor.matmul(
                    r_ps[:, 0:w],
                    ones65[64:65, :],
                    rcb[64:65, 0:w],
                    start=True,
                    stop=True,
                )
                hs = (0, 1) if kind == "pair" else (2,)
                for i, h in enumerate(hs):
                    nc.vector.tensor_tensor(
                        attnT[h][:, qsl],
                        an[0:64, i * 256 : (i + 1) * 256],
                        r_ps[:, i * 256 : (i + 1) * 256],
                        ALU.mult,
                    )

            # pipeline state
            pend_pv = []    # (a_ps, pe_t, pv_ops, pv_first, norm_args)
            pend_norm = []  # (due_batch, norm_args)
            batch_no = [0]

            def flush_pv(keep):
                while len(pend_pv) > keep:
                    a_ps, pe_t, ops, pv_first, norm_after = pend_pv.pop(0)
                    for h, kt, j, pc, oc, n, last in ops:
                        first = (kt == 0) and not pv_first
                        m = nc.tensor.matmul(
                            a_ps[0:65, oc : oc + n],
                            vaug[:, kt, h, 0:65],
                            pe_t[:, j, pc : pc + n],
                            start=first,
                            stop=last,
                            skip_group_check=not first,
                        )
                        if first:
                            pv_first.append(m)
                        elif kt == 0 and not _NODEPS:
                            add_dep_helper(m.ins, pv_first[0].ins, False, "aps order")
                    if norm_after is not None:
                        pend_norm.append(
                            (batch_no[0] + 4, start_norm(*norm_after))
                        )

            def flush_norms(force=False):
                while pend_norm and (force or pend_norm[0][0] <= batch_no[0]):
                    _, args = pend_norm.pop(0)
                    finish_norm(*args)

            def emit_phaseD(ts, half=None):
                # half: 0/1 selects a 256-query half of the ts chunk (used to
                # pull the s=6 half of the final chunk out of the tail)
                if half is None:
                    tsl = slice(ts * 512, (ts + 1) * 512)
                    w = 512
                else:
                    tsl = slice(ts * 512 + half * 256, ts * 512 + (half + 1) * 256)
                    w = 256
                for oc in range(C // 128):
                    ocs = slice(oc * 128, (oc + 1) * 128)
                    po = aux.tile([128, 512], F32, tag="aux", name="po")
                    for h in range(HPG):
                        nc.tensor.matmul(
                            po[:, 0:w],
                            wo_sb[h][:, ocs],
                            attnT[h][:, tsl],
                            start=(h == 0),
                            stop=(h == HPG - 1),
                        )
                    ob = ob_pool.tile([128, 512], BF16, tag="ob")
                    # copy via ScalarE: mid-kernel it has slack, and in the
                    # tail the exp stream is already done while DVE paces
                    with nc.allow_low_precision(reason="bf16 output cast"):
                        nc.scalar.copy(ob[:, 0:w], po[:, 0:w])
                    nc.sync.dma_start(out[ocs, tsl], ob[:, 0:w])

            def run_batch(s, kind, a_ps, pv_first, sc_ops, tail, exp_w,
                          pv_ops, norm_after):
                """sc_ops: (h, kt, bank, col, q0, n, rowbase)
                   pv_ops: (h, kt, bank, pecol, outcol, n, last)"""
                bs = sps.tile([128, BK, 512], F32, tag="s")
                bank_first = {}
                for h, kt, j, c0, q0, n, rb in sc_ops:
                    first = j not in bank_first
                    m = nc.tensor.matmul(
                        bs[:, j, c0 : c0 + n],
                        s_lhsT(h, kt, rb), s_rhs(h, s, q0, n, rb),
                        start=first, stop=True,
                        skip_group_check=not first,
                    )
                    if first:
                        bank_first[j] = m
                    elif not _NODEPS:
                        add_dep_helper(m.ins, bank_first[j].ins, False, "bank order")
                batch_no[0] += 1
                flush_pv(LAG)
                flush_norms()
                pe_t = pepool.tile([128, BK, 512], BF16, tag="pe")
                nc.scalar.activation(
                    pe_t[:, :, 0:exp_w], bs[:, :, 0:exp_w], AF.Exp, scale=0.125
                )
                if tail:
                    # multiplicative 0/1 tail mask on DVE (keeps masking off
                    # the PE; diag + invalid blocks zeroed post-exp)
                    nc.vector.tensor_tensor(
                        pe_t[:, :, 0:384], pe_t[:, :, 0:384], tm_sb[:],
                        ALU.mult,
                    )
                pend_pv.append((a_ps, pe_t, pv_ops, pv_first, norm_after))

            def attn_unit(s, kind):
                nkt_steady = 4 * s
                t0 = 4 * s  # first tail kt
                flush_norms(force=True)
                if kind == "solo" and s >= 2 and s % 2 == 0:
                    emit_phaseD((s - 2) // 2)
                if kind == "solo" and s == 7:
                    # s=6 norms are flushed by now: the s=6 half of the final
                    # output chunk can overlap s=7's attention work
                    emit_phaseD(3, half=0)
                w_aps = 512 if kind == "pair" else 256
                a_ps = apsp.tile([65, w_aps], F32, tag="attn", name="a_ps")
                pv_first = []

                if kind == "pair":
                    for kt0 in range(0, nkt_steady, 2):
                        sc, pv = [], []
                        for i, kt in enumerate((kt0, kt0 + 1)):
                            for h in (0, 1):
                                sc.append((h, kt, h, i * 256, 0, 256, 0))
                                pv.append((h, kt, h, i * 256, h * 256, 256, False))
                        run_batch(s, kind, a_ps, pv_first, sc, False, 512, pv, None)
                    # tail: one tile per head; bank0=[kt0 q256|kt0+2 j1],
                    # bank1=[kt0+1 q256|kt0+3 j1]
                    for h in (0, 1):
                        sc = [
                            (h, t0, 0, 0, 0, 256, 0),
                            (h, t0 + 1, 1, 0, 0, 256, 0),
                            (h, t0 + 2, 0, 256, 128, 128, 0),
                            (h, t0 + 3, 1, 256, 128, 128, 0),
                        ]
                        # kt4s+1 last with stop=True: its 256-wide write is
                        # the final touch on every element of this head's
                        # a_ps region, closing the accumulation group.
                        oc = h * 256
                        pv = [
                            (h, t0, 0, 0, oc, 256, False),
                            (h, t0 + 2, 0, 256, oc + 128, 128, False),
                            (h, t0 + 3, 1, 256, oc + 128, 128, False),
                            (h, t0 + 1, 1, 0, oc, 256, True),
                        ]
                        norm = ("pair", s, a_ps) if h == 1 else None
                        run_batch(s, kind, a_ps, pv_first, sc, True, 384, pv, norm)
                else:
                    for kt0 in range(0, nkt_steady, 4):
                        sc, pv = [], []
                        for d, kt in enumerate(range(kt0, kt0 + 4)):
                            j, c0 = d & 1, (d // 2) * 256
                            rb = 0 if _NOROWTILE else 64 * (kt % 2)
                            sc.append((2, kt, j, c0, 0, 256, rb))
                            pv.append((2, kt, j, c0, 0, 256, False))
                        run_batch(s, kind, a_ps, pv_first, sc, False, 512, pv, None)
                    rbo = 0 if _NOROWTILE else 64
                    sc = [
                        (2, t0, 0, 0, 0, 256, 0),
                        (2, t0 + 1, 1, 0, 0, 256, rbo),
                        (2, t0 + 2, 0, 256, 128, 128, 0),
                        (2, t0 + 3, 1, 256, 128, 128, rbo),
                    ]
                    pv = [
                        (2, t0, 0, 0, 0, 256, False),
                        (2, t0 + 2, 0, 256, 128, 128, False),
                        (2, t0 + 3, 1, 256, 128, 128, False),
                        (2, t0 + 1, 1, 0, 0, 256, True),
                    ]
                    run_batch(s, kind, a_ps, pv_first, sc, True, 384, pv,
                              ("solo", s, a_ps))

            unit_list = []
            for s in range(NS):
                unit_list.append(("kv", s))
                if s == 0:
                    unit_list.append(("q", 0))
                unit_list.append((s, "pair"))
                unit_list.append((s, "solo"))
                if s % 2 == 1 and s < 7:
                    unit_list.append(("q", (s + 1) // 2))

            for s, kind in unit_list:
                if s == "kv":
                    kv_chunk(kind)
                    continue
                if s == "q":
                    q_chunk(kind)
                    continue
                attn_unit(s, kind)
            flush_pv(0)
            flush_norms(force=True)
            emit_phaseD(3, half=1)

    nc.compile()
    return nc


def _get_nc():
    if "nc" not in _CACHE:
        _CACHE["nc"] = build_nc()
    return _CACHE["nc"]


BF = ml_dtypes.bfloat16


def pack_w(w):
    """[C, n] -> [128, KO*n] so each SBUF partition row is contiguous."""
    n = w.shape[1]
    return np.ascontiguousarray(
        w.reshape(KO, 128, n).transpose(1, 0, 2).reshape(128, KO * n)
    ).astype(BF)


def make_in_maps(inputs):
    """Shard full inputs into 8 per-core input maps.

    tmask [128k, 2, 384]: per-parity additive masks for the two tail
    banks.  Bank A covers [kt4s j0 | kt4s j1 | kt4s+2 j1], bank B covers
    [kt4s+1 j0 | kt4s+1 j1 | kt4s+3 j1].
      qh=0: A = [diag | 0 | diag],  B = [-inf | 0 | -inf]
      qh=1: A = [0    | 0 | 0   ],  B = [diag | 0 | diag]
    """
    x = np.ascontiguousarray(np.asarray(inputs["x"], dtype=np.float32)).reshape(T, C)
    W_qkv = np.asarray(inputs["W_qkv"], dtype=np.float32)
    W_out = np.asarray(inputs["W_out"], dtype=np.float32)

    # multiplicative masks: keep = 1, drop = 0; [k, q]: keep q >= k
    diag_keep = np.where(
        np.arange(128)[None, :] >= np.arange(128)[:, None],
        np.float32(1), np.float32(0),
    )

    # chunk-major layouts: [128, chunk, KO, 512]
    xT = np.ascontiguousarray(
        x.T.reshape(KO, 128, 8, 512).transpose(1, 2, 0, 3)
    ).astype(BF)
    xr = x.reshape(NTT, 128, C)
    xqT = {
        qh: np.ascontiguousarray(
            xr[qh::2].reshape(TQ, C).T.reshape(KO, 128, 4, 512).transpose(1, 2, 0, 3)
        ).astype(BF)
        for qh in (0, 1)
    }

    ZK = np.zeros((128, 128), np.float32)   # fully-masked block
    ONE = np.ones((128, 128), np.float32)   # fully-valid block
    tmask = {}
    for qh in (0, 1):
        if qh == 0:
            A = np.concatenate([diag_keep, ONE, diag_keep], axis=1)
            B = np.concatenate([ZK, ONE, ZK], axis=1)
        else:
            A = np.concatenate([ONE, ONE, ONE], axis=1)
            B = np.concatenate([diag_keep, ONE, diag_keep], axis=1)
        tmask[qh] = np.stack([A, B], axis=1).astype(BF)  # [128, 2, 384]

    in_maps = []
    for c in range(N_CORES):
        g, qh = c // 2, c % 2
        in_maps.append(
            {
                "xT": xT,
                "xqT": xqT[qh],
                "wpack": pack_w(
                    np.concatenate(
                        [
                            W_qkv[:, 1 * C + g * GCH : 1 * C + (g + 1) * GCH],
                            W_qkv[:, 2 * C + g * GCH : 2 * C + (g + 1) * GCH],
                            W_qkv[:, 0 * C + g * GCH : 0 * C + (g + 1) * GCH],
                        ],
                        axis=1,
                    )
                ),
                "wo": np.ascontiguousarray(W_out[g * GCH : (g + 1) * GCH, :]).astype(BF),
                "tmask": tmask[qh],
            }
        )
    return in_maps


def combine_outputs(parts, b_out):
    """Sum head-group partials per parity, reassemble rows, add bias."""
    NQT = TQ // 128
    out = np.zeros((T, C), np.float32)
    orow = out.reshape(NTT, 128, C)
    for qh in (0, 1):
        acc = parts[qh].astype(np.float32).copy()
        for g in range(1, 4):
            acc += parts[2 * g + qh]
        orow[qh::2] = np.ascontiguousarray(acc.T).reshape(NQT, 128, C)
    out += np.asarray(b_out, dtype=np.float32)[None, :]
    return out.reshape(1, T, C)


def _run(inputs, trace=False, tmpdir=None):
    nc = _get_nc()
    in_maps = make_in_maps(inputs)
    res = bass_utils.run_bass_kernel_spmd(
        nc, in_maps, core_ids=list(range(N_CORES)), trace=trace, tmpdir=tmpdir
    )
    parts = [np.asarray(res.results[c]["out"]) for c in range(N_CORES)]
    return combine_outputs(parts, inputs["b_out"]), res


def kernel(**inputs):
    out, _ = _run(inputs)
    return out
